# revision 1
# baseline (speedup 1.0000x reference)
"""Multi-head cross-attention (B=4, S=2048, D=1024, H=16) on 8 Trainium2 cores.

Sharding: hybrid data/tensor parallel. Core c handles batch b = c//2 and
head-group g = c%2 (8 of the 16 heads, i.e. 512 of the 1024 q/k/v dims).
Each core computes a partial out-projection over its 512 attention dims;
the host sums the two partials per batch (the "all-reduce after out_lin"
of the tensor-parallel split, done on host since pairs share a batch).

Per-core kernel (all matmuls in float32r = rounded-fp32 PE fast path):
  Q.T = wq_t.T @ x_t   (+bq)      [512, 2048]   (o on partitions)
  K.T = wk_t.T @ mem_t (+bk)      [512, 2048]
  V   = mem_t.T @ wv_t            [2048, 512] stored as v_aug [*, 8, 65]
                                  with a ones column per head (denominator)
  per head h, query-half qh:
    S.T[k,q] = K_h @ Q_h.T                (K=64 contraction)
    P.T      = exp(0.125*S.T + mask_bias) (ACT, bias is per-key partition)
    AV.T     = [V_h|1].T @ P.T  -> [65, 1024] PSUM accum over k-chunks
    attn.T   = AV.T[0:64] * recip(AV.T[64])  (Newton-refined reciprocal)
  out.T = wo_t.T @ attn.T (+bo_eff on core g=0)  [1024, 2048] partial

bv is folded into bo on the host: out = attn@wo.T + (bo + wo@bv) because
softmax rows sum to 1. The key-padding mask enters as an additive
per-partition bias in the exp activation (exact, and free).
"""

import numpy as np

import concourse.bacc as bacc
import concourse.mybir as mybir
from concourse import tile
from concourse.bass_utils import run_bass_kernel_spmd

F32 = mybir.dt.float32
F32R = mybir.dt.float32r
F16 = mybir.dt.float16
AF = mybir.ActivationFunctionType

B, S, D = 4, 2048, 1024
H, HD = 16, 64
NCORES = 8
NH = 8          # heads per core
OD = NH * HD    # 512 attention dims per core
P = 128
NDC = D // P    # 8 d-chunks
NKC = S // P    # 16 key chunks
NEG = -1.0e30

_cache = {}


def _build():
    from contextlib import ExitStack

    nc = bacc.Bacc(None, target_bir_lowering=False, debug=False)

    x_t = nc.dram_tensor("x_t", [D, S], F16, kind="ExternalInput").ap()
    mem_t = nc.dram_tensor("mem_t", [D, S], F16, kind="ExternalInput").ap()
    wq_t = nc.dram_tensor("wq_t", [D, OD], F16, kind="ExternalInput").ap()
    wk_t = nc.dram_tensor("wk_t", [D, OD], F16, kind="ExternalInput").ap()
    wv_t = nc.dram_tensor("wv_t", [D, OD], F16, kind="ExternalInput").ap()
    wo_t = nc.dram_tensor("wo_t", [OD, D], F16, kind="ExternalInput").ap()
    bq_s = nc.dram_tensor("bq_s", [P, OD // P], F32, kind="ExternalInput").ap()
    bk_s = nc.dram_tensor("bk_s", [P, OD // P], F32, kind="ExternalInput").ap()
    bo_s = nc.dram_tensor("bo_s", [P, D // P], F32, kind="ExternalInput").ap()
    maskb = nc.dram_tensor("maskb", [P, NKC], F32, kind="ExternalInput").ap()
    out_t = nc.dram_tensor("out_t", [D, S], F32, kind="ExternalOutput").ap()

    x_c = x_t.rearrange("(c p) s -> c p s", p=P)
    m_c = mem_t.rearrange("(c p) s -> c p s", p=P)
    wq_c = wq_t.rearrange("(c p) o -> c p o", p=P)
    wk_c = wk_t.rearrange("(c p) o -> c p o", p=P)
    wv_c = wv_t.rearrange("(c p) o -> c p o", p=P)
    wo_c = wo_t.rearrange("(c p) o -> c p o", p=P)

    with tile.TileContext(nc) as tc, ExitStack() as ctx:
        q_pool = ctx.enter_context(tc.tile_pool(name="qt", bufs=1))
        k_pool = ctx.enter_context(tc.tile_pool(name="kt", bufs=1))
        v_pool = ctx.enter_context(tc.tile_pool(name="va", bufs=1))
        a_pool = ctx.enter_context(tc.tile_pool(name="at", bufs=1))
        c_pool = ctx.enter_context(tc.tile_pool(name="cst", bufs=1))
        w_pool = ctx.enter_context(tc.tile_pool(name="wt", bufs=10))
        e_pool = ctx.enter_context(tc.tile_pool(name="es", bufs=4))
        n_pool = ctx.enter_context(tc.tile_pool(name="nrm", bufs=2))
        o_pool = ctx.enter_context(tc.tile_pool(name="ev", bufs=2))
        psum_pool = ctx.enter_context(tc.tile_pool(name="ps", bufs=2, space="PSUM"))
        xm_pool = ctx.enter_context(tc.tile_pool(name="xm", bufs=8))

        # ---- constants and first-needed weights before the bulk loads ----
        bq_sb = c_pool.tile([P, OD // P], F32, tag="bq")
        bk_sb = c_pool.tile([P, OD // P], F32, tag="bk")
        bo_sb = c_pool.tile([P, D // P], F32, tag="bo")
        mk_sb = c_pool.tile([P, NKC], F32, tag="mk")
        nc.sync.dma_start(out=bq_sb[:], in_=bq_s[:])
        nc.sync.dma_start(out=bk_sb[:], in_=bk_s[:])
        nc.sync.dma_start(out=bo_sb[:], in_=bo_s[:])
        nc.sync.dma_start(out=mk_sb[:], in_=maskb[:])
        ones_f = c_pool.tile([P, NH], F32, tag="onef")
        nc.vector.memset(ones_f[:], 1.0)
        ones_r = c_pool.tile([P, NH], F16, tag="oner")
        nc.vector.tensor_copy(ones_r[:], ones_f[:])
        wk0_tiles = []
        for i in range(NDC):
            wt = w_pool.tile([P, P], F16, tag="w", name="wk0", bufs=10)
            nc.sync.dma_start(out=wt[:], in_=wk_c[i, :, 0:P])
            wk0_tiles.append(wt)

        # ---- persistent tiles ----
        qT = [q_pool.tile([P, S], F16, tag=f"q{m}", name=f"q{m}")
              for m in range(OD // P)]
        kT = [k_pool.tile([P, S], F16, tag=f"k{h}", name=f"k{h}")
              for h in range(NH)]
        for h in range(NH):
            ro = 64 * (h % 2)
            nc.vector.memset(kT[h][64 - ro:128 - ro, :], 0.0)
        v_aug = [v_pool.tile([P, 9, 65], F16, tag=f"v{st}", name=f"v{st}")
                 for st in range(NKC)]
        for st in range(NKC):
            nc.vector.memset(v_aug[st][:, 8, :], 0.0)
        attn = [a_pool.tile([P, S], F16, tag=f"a{m}", name=f"a{m}")
                for m in range(OD // P)]

        # ---- K.T projection (all head-pairs) ----
        m_tiles = []
        for i in range(NDC):
            t = xm_pool.tile([P, S], F16, tag="xm", name="mt")
            eng = nc.sync if i % 2 == 0 else nc.gpsimd
            eng.dma_start(out=t[:], in_=m_c[i])
            m_tiles.append(t)
        for m in range(OD // P):
            wk_tiles = wk0_tiles if m == 0 else []
            if m > 0:
                for i in range(NDC):
                    wt = w_pool.tile([P, P], F16, tag="w", name="wkt", bufs=10)
                    nc.sync.dma_start(out=wt[:], in_=wk_c[i, :, m * P:(m + 1) * P])
                    wk_tiles.append(wt)
            for n in range(2):
                csl = slice(n * 1024, (n + 1) * 1024)
                ps = psum_pool.tile([P, 1024], F32, tag="lg", name="psk")
                for i in range(NDC):
                    for j in range(2):
                        nc.tensor.matmul(
                            ps[:, j * 512:(j + 1) * 512], wk_tiles[i][:],
                            m_tiles[i][:, n * 1024 + j * 512:
                                       n * 1024 + (j + 1) * 512],
                            start=(i == 0), stop=(i == NDC - 1),
                        )
                nc.vector.tensor_scalar_add(
                    kT[2 * m][0:64, csl], ps[0:64, :], bk_sb[0:64, m:m + 1])
                nc.scalar.activation(
                    kT[2 * m + 1][64:128, csl], ps[64:128, :], AF.Identity,
                    bias=bk_sb[64:128, m:m + 1])

        # ---- V into v_aug ----
        wv_tiles = []
        for i in range(NDC):
            wt = w_pool.tile([P, OD], F16, tag="wv", name="wvt", bufs=8)
            nc.sync.dma_start(out=wt[:], in_=wv_c[i])
            wv_tiles.append(wt)
        for st in range(NKC):
            ps = psum_pool.tile([P, 1024], F32, tag="lg", name="psv")
            for i in range(NDC):
                nc.tensor.matmul(
                    ps[:, 0:OD], m_tiles[i][:, st * P:(st + 1) * P],
                    wv_tiles[i][:],
                    start=(i == 0), stop=(i == NDC - 1),
                )
            nc.vector.tensor_copy(
                v_aug[st][:, 0:NH, 0:64],
                ps[:, 0:OD].rearrange("p (h d) -> p h d", h=NH),
            )
            nc.vector.tensor_copy(
                v_aug[st][:, 0:NH, 64:65], ones_r[:].unsqueeze(2))

        # ---- x loads (reuse xm slots) ----
        x_tiles = []
        for i in range(NDC):
            t = xm_pool.tile([P, S], F16, tag="xm", name="xt")
            eng = nc.sync if i % 2 == 0 else nc.gpsimd
            eng.dma_start(out=t[:], in_=x_c[i])
            x_tiles.append(t)

        # ---- per head-pair: Q.T projection, then attention ----
        for mt in range(OD // P):
            wq_tiles = []
            for i in range(NDC):
                wt = w_pool.tile([P, P], F16, tag="w", name="wqt", bufs=10)
                nc.sync.dma_start(out=wt[:], in_=wq_c[i, :, mt * P:(mt + 1) * P])
                wq_tiles.append(wt)
            for n in range(2):
                csl = slice(n * 1024, (n + 1) * 1024)
                ps = psum_pool.tile([P, 1024], F32, tag="lg", name="psq")
                for i in range(NDC):
                    for j in range(2):
                        nc.tensor.matmul(
                            ps[:, j * 512:(j + 1) * 512], wq_tiles[i][:],
                            x_tiles[i][:, n * 1024 + j * 512:
                                       n * 1024 + (j + 1) * 512],
                            start=(i == 0), stop=(i == NDC - 1),
                        )
                nc.scalar.activation(
                    qT[mt][:, csl], ps[:], AF.Identity,
                    bias=bq_sb[:, mt:mt + 1])

            for h in (2 * mt, 2 * mt + 1):
                ro = 64 * (h % 2)
                for qh in range(2):
                    q_sl = slice(qh * 1024, (qh + 1) * 1024)
                    av = psum_pool.tile([P, 1024], F32, tag="av", name="av")
                    for kc in range(NKC):
                        lg = psum_pool.tile([P, 1024], F32, tag="lg", name="lg")
                        for j in range(2):
                            nc.tensor.matmul(
                                lg[:, j * 512:(j + 1) * 512],
                                kT[h][:, kc * P:(kc + 1) * P],
                                qT[mt][:, qh * 1024 + j * 512:
                                        qh * 1024 + (j + 1) * 512],
                                start=True, stop=True,
                            )
                        es = e_pool.tile([P, 1024], F16, tag="es")
                        nc.scalar.activation(
                            es[:], lg[:], AF.Exp,
                            bias=mk_sb[:, kc:kc + 1], scale=0.125,
                        )
                        va_flat = v_aug[kc][:].rearrange("p h d -> p (h d)")
                        for j in range(2):
                            nc.tensor.matmul(
                                av[:, j * 512:(j + 1) * 512],
                                va_flat[:, 65 * h:65 * h + 128],
                                es[:, j * 512:(j + 1) * 512],
                                start=(kc == 0), stop=(kc == NKC - 1),
                            )
                    r0 = n_pool.tile([1, 1024], F32, tag="r0")
                    bc = n_pool.tile([64, 1024], F32, tag="bc")
                    nc.vector.reciprocal(r0[:], av[64:65, :])
                    nc.gpsimd.partition_broadcast(bc[:], r0[:])
                    nc.vector.tensor_mul(
                        attn[mt][ro:ro + 64, q_sl], av[0:64, :], bc[:])

        # ---- out.T = wo_t.T @ attn.T (+bo_eff) ----
        for m in range(D // P):
            wo_tiles = []
            for i in range(OD // P):
                wt = w_pool.tile([P, P], F16, tag="w", name="wot", bufs=10)
                nc.sync.dma_start(out=wt[:], in_=wo_c[i, :, m * P:(m + 1) * P])
                wo_tiles.append(wt)
            for n in range(2):
                csl = slice(n * 1024, (n + 1) * 1024)
                ps = psum_pool.tile([P, 1024], F32, tag="av", name="pso")
                for i in range(OD // P):
                    for j in range(2):
                        nc.tensor.matmul(
                            ps[:, j * 512:(j + 1) * 512], wo_tiles[i][:],
                            attn[i][:, n * 1024 + j * 512:
                                    n * 1024 + (j + 1) * 512],
                            start=(i == 0), stop=(i == OD // P - 1),
                        )
                ev = o_pool.tile([P, 1024], F32, tag="ev")
                if (2 * m + n) % 2 == 0:
                    nc.vector.tensor_scalar_add(ev[:], ps[:], bo_sb[:, m:m + 1])
                else:
                    nc.scalar.activation(
                        ev[:], ps[:], AF.Identity, bias=bo_sb[:, m:m + 1])
                nc.sync.dma_start(out=out_t[m * P:(m + 1) * P, csl], in_=ev[:])

    nc.compile()
    return nc


def _prep_inputs(x, memory, mask, wq, bq, wk, bk, wv, bv, wo, bo):
    f = np.float32
    h = np.float16
    wqT = np.ascontiguousarray(wq.T, dtype=f)
    wkT = np.ascontiguousarray(wk.T, dtype=f)
    wvT = np.ascontiguousarray(wv.T, dtype=f)
    woT = np.ascontiguousarray(wo.T, dtype=f)
    bo_eff = (bo.astype(f) + wo.astype(f) @ bv.astype(f))
    zeros_bo = np.zeros_like(bo_eff)
    in_maps = []
    for c in range(NCORES):
        b, g = divmod(c, 2)
        sl = slice(g * OD, (g + 1) * OD)
        bo_c = bo_eff if g == 0 else zeros_bo
        in_maps.append({
            "x_t": np.ascontiguousarray(x[b].T, dtype=h),
            "mem_t": np.ascontiguousarray(memory[b].T, dtype=h),
            "wq_t": np.ascontiguousarray(wqT[:, sl]).astype(h),
            "wk_t": np.ascontiguousarray(wkT[:, sl]).astype(h),
            "wv_t": np.ascontiguousarray(wvT[:, sl]).astype(h),
            "wo_t": np.ascontiguousarray(woT[sl, :]).astype(h),
            "bq_s": np.ascontiguousarray(bq[sl].astype(f).reshape(OD // P, P).T),
            "bk_s": np.ascontiguousarray(bk[sl].astype(f).reshape(OD // P, P).T),
            "bo_s": np.ascontiguousarray(bo_c.reshape(D // P, P).T),
            "maskb": np.ascontiguousarray(
                np.where(mask[b], np.float32(NEG), np.float32(0.0))
                .astype(f).reshape(NKC, P).T),
        })
    return in_maps


def kernel(x, memory, mask, wq, bq, wk, bk, wv, bv, wo, bo, **run_kwargs):
    x = np.asarray(x, dtype=np.float32)
    memory = np.asarray(memory, dtype=np.float32)
    mask = np.asarray(mask)
    if "nc" not in _cache:
        _cache["nc"] = _build()
    nc = _cache["nc"]
    in_maps = _prep_inputs(x, memory, mask, wq, bq, wk, bk, wv, bv, wo, bo)
    res = run_bass_kernel_spmd(nc, in_maps, list(range(NCORES)), **run_kwargs)
    out = np.empty((B, S, D), dtype=np.float32)
    for b in range(B):
        part = res.results[2 * b]["out_t"] + res.results[2 * b + 1]["out_t"]
        out[b] = part.T
    if run_kwargs:
        _cache["last_results"] = res
    return out



# revision 7
# speedup vs baseline: 1.0503x; 1.0503x over previous
"""Multi-head cross-attention (B=4, S=2048, D=1024, H=16) on 8 Trainium2 cores.

Sharding: hybrid data/tensor parallel. Core c handles batch b = c//2 and
head-group g = c%2 (8 of the 16 heads, i.e. 512 of the 1024 q/k/v dims).
Each core computes a partial out-projection over its 512 attention dims;
the host sums the two partials per batch (the "all-reduce after out_lin"
of the tensor-parallel split, done on host since pairs share a batch).

Per-core kernel:
  Q.T = wq_t.T @ x_t   (+bq)      [512, 2048]
  K.T = wk_t.T @ mem_t (+bk)      [512, 2048]
  V   = mem_t.T @ wv_t            [2048, 512] stored as v_aug [*, 9, 65]
                                  with a ones column per head (denominator)
  per head h, query-half qh:
    S.T[k,q] = K_h @ Q_h.T                (K=64 contraction, padded 128)
    P.T      = exp(0.125*S.T + mask_bias) (ACT, bias is per-key partition)
    AV.T     = [V_h|1].T @ P.T  -> [65, 1024] PSUM accum over k-chunks
    attn.T   = AV.T[0:64] * recip_approx(AV.T[64])
  out.T = wo_t.T @ attn.T (+bo_eff on core g=0)  [1024, 2048] partial, f16

bv is folded into bo on the host: out = attn@wo.T + (bo + wo@bv) because
softmax rows sum to 1. The key-padding mask enters as an additive
per-partition bias in the exp activation (exact, and free).
"""

import numpy as np

import concourse.bacc as bacc
import concourse.mybir as mybir
from concourse import tile
from concourse.bass_utils import run_bass_kernel_spmd

F32 = mybir.dt.float32
F16 = mybir.dt.float16
AF = mybir.ActivationFunctionType

B, S, D = 4, 2048, 1024
H, HD = 16, 64
NCORES = 8
NH = 8          # heads per core
OD = NH * HD    # 512 attention dims per core
P = 128
NDC = D // P    # 8 d-chunks
NKC = S // P    # 16 key chunks
NEG = -1.0e30

_cache = {}


def _build():
    from contextlib import ExitStack

    nc = bacc.Bacc(None, target_bir_lowering=False, debug=False)

    x_t = nc.dram_tensor("x_t", [D, S], F16, kind="ExternalInput").ap()
    mem_t = nc.dram_tensor("mem_t", [D, S], F16, kind="ExternalInput").ap()
    wq_t = nc.dram_tensor("wq_t", [D, OD], F16, kind="ExternalInput").ap()
    wk_t = nc.dram_tensor("wk_t", [D, OD], F16, kind="ExternalInput").ap()
    wv_t = nc.dram_tensor("wv_t", [D, OD], F16, kind="ExternalInput").ap()
    wo_t = nc.dram_tensor("wo_t", [OD, D], F16, kind="ExternalInput").ap()
    bq_s = nc.dram_tensor("bq_s", [P, OD // P], F32, kind="ExternalInput").ap()
    bk_s = nc.dram_tensor("bk_s", [P, OD // P], F32, kind="ExternalInput").ap()
    bo_s = nc.dram_tensor("bo_s", [P, D // P], F32, kind="ExternalInput").ap()
    maskb = nc.dram_tensor("maskb", [P, NKC], F32, kind="ExternalInput").ap()
    out_t = nc.dram_tensor("out_t", [D, S], F16, kind="ExternalOutput").ap()

    x_c = x_t.rearrange("(c p) s -> c p s", p=P)
    m_c = mem_t.rearrange("(c p) s -> c p s", p=P)
    wq_c = wq_t.rearrange("(c p) o -> c p o", p=P)
    wk_c = wk_t.rearrange("(c p) o -> c p o", p=P)
    wv_c = wv_t.rearrange("(c p) o -> c p o", p=P)
    wo_c = wo_t.rearrange("(c p) o -> c p o", p=P)

    with tile.TileContext(nc) as tc, ExitStack() as ctx:
        q_pool = ctx.enter_context(tc.tile_pool(name="qt", bufs=1))
        k_pool = ctx.enter_context(tc.tile_pool(name="kt", bufs=1))
        v_pool = ctx.enter_context(tc.tile_pool(name="va", bufs=1))
        a_pool = ctx.enter_context(tc.tile_pool(name="at", bufs=1))
        c_pool = ctx.enter_context(tc.tile_pool(name="cst", bufs=1))
        w_pool = ctx.enter_context(tc.tile_pool(name="wt", bufs=1))
        e_pool = ctx.enter_context(tc.tile_pool(name="es", bufs=6))
        n_pool = ctx.enter_context(tc.tile_pool(name="nrm", bufs=2))
        o_pool = ctx.enter_context(tc.tile_pool(name="ev", bufs=2))
        psum_pool = ctx.enter_context(tc.tile_pool(name="ps", bufs=2, space="PSUM"))
        xm_pool = ctx.enter_context(tc.tile_pool(name="xm", bufs=16))

        # ---- weight / input DMAs: spread across queues, first-needed first
        wk_sb = [w_pool.tile([P, OD], F16, tag="wk", name=f"wk{i}", bufs=NDC)
                 for i in range(NDC)]
        for i in range(NDC):
            nc.sync.dma_start(out=wk_sb[i][:], in_=wk_c[i])

        # mem tiles, n-split for finer dependencies and queue parallelism
        m_sb = [[xm_pool.tile([P, 1024], F16, tag="xm", name=f"m{n}_{i}")
                 for i in range(NDC)] for n in range(2)]
        for i in range(NDC):
            nc.gpsimd.dma_start(out=m_sb[0][i][:], in_=m_c[i, :, 0:1024])
        bq_sb = c_pool.tile([P, OD // P], F32, tag="bq")
        bk_sb = c_pool.tile([P, OD // P], F32, tag="bk")
        bo_sb = c_pool.tile([P, D // P], F32, tag="bo")
        mk_sb = c_pool.tile([P, NKC], F32, tag="mk")
        nc.scalar.dma_start(out=bk_sb[:], in_=bk_s[:])
        for i in range(NDC):
            nc.scalar.dma_start(out=m_sb[1][i][:], in_=m_c[i, :, 1024:2048])
        nc.scalar.dma_start(out=bq_sb[:], in_=bq_s[:])
        nc.scalar.dma_start(out=mk_sb[:], in_=maskb[:])
        nc.scalar.dma_start(out=bo_sb[:], in_=bo_s[:])

        wv_sb = [w_pool.tile([P, OD], F16, tag="wv", name=f"wv{i}", bufs=NDC)
                 for i in range(NDC)]
        for i in range(NDC):
            nc.sync.dma_start(out=wv_sb[i][:], in_=wv_c[i])
        wq_sb = [w_pool.tile([P, OD], F16, tag="wq", name=f"wq{i}", bufs=NDC)
                 for i in range(NDC)]
        for i in range(NDC):
            nc.sync.dma_start(out=wq_sb[i][:], in_=wq_c[i])
        wo_sb = [w_pool.tile([P, D], F16, tag="wo", name=f"wo{i}", bufs=OD // P)
                 for i in range(OD // P)]
        for i in range(OD // P):
            nc.sync.dma_start(out=wo_sb[i][:], in_=wo_c[i])

        # x tiles, n-split
        x_sb = [[xm_pool.tile([P, 1024], F16, tag="xm", name=f"x{n}_{i}")
                 for i in range(NDC)] for n in range(2)]
        for n in range(2):
            for i in range(NDC):
                nc.gpsimd.dma_start(out=x_sb[n][i][:],
                                    in_=x_c[i, :, n * 1024:(n + 1) * 1024])

        # ---- persistent tiles ----
        qT = [q_pool.tile([P, S], F16, tag=f"q{m}", name=f"q{m}")
              for m in range(OD // P)]
        kT = [k_pool.tile([P, S], F16, tag=f"k{h}", name=f"k{h}")
              for h in range(NH)]
        for h in range(NH):
            ro = 64 * (h % 2)
            nc.vector.memset(kT[h][64 - ro:128 - ro, :], 0.0)
        ones_f = c_pool.tile([P, NH], F32, tag="onef")
        nc.vector.memset(ones_f[:], 1.0)
        ones_r = c_pool.tile([P, NH], F16, tag="oner")
        nc.vector.tensor_copy(ones_r[:], ones_f[:])
        v_aug = [v_pool.tile([P, 9, 65], F16, tag=f"v{st}", name=f"v{st}")
                 for st in range(NKC)]
        for st in range(NKC):
            nc.vector.memset(v_aug[st][:, 8, :], 0.0)
        attn = [a_pool.tile([P, S], F16, tag=f"a{m}", name=f"a{m}")
                for m in range(OD // P)]

        # ---- K.T projection (all head-pairs) ----
        for m in range(OD // P):
            for n in range(2):
                csl = slice(n * 1024, (n + 1) * 1024)
                ps = psum_pool.tile([P, 1024], F32, tag="lg", name="psk")
                for i in range(NDC):
                    for j in range(2):
                        nc.tensor.matmul(
                            ps[:, j * 512:(j + 1) * 512],
                            wk_sb[i][:, m * P:(m + 1) * P],
                            m_sb[n][i][:, j * 512:(j + 1) * 512],
                            start=(i == 0), stop=(i == NDC - 1),
                        )
                nc.vector.tensor_scalar_add(
                    kT[2 * m][0:64, csl], ps[0:64, :], bk_sb[0:64, m:m + 1])
                nc.vector.tensor_scalar_add(
                    kT[2 * m + 1][64:128, csl], ps[64:128, :],
                    bk_sb[64:128, m:m + 1])

        # ---- V into v_aug ----
        for st in range(NKC):
            n, sc = divmod(st, 8)
            ps = psum_pool.tile([P, 1024], F32, tag="lg", name="psv")
            for i in range(NDC):
                nc.tensor.matmul(
                    ps[:, 0:OD], m_sb[n][i][:, sc * P:(sc + 1) * P],
                    wv_sb[i][:],
                    start=(i == 0), stop=(i == NDC - 1),
                )
            nc.vector.tensor_copy(
                v_aug[st][:, 0:NH, 0:64],
                ps[:, 0:OD].rearrange("p (h d) -> p h d", h=NH),
            )
            nc.vector.tensor_copy(
                v_aug[st][:, 0:NH, 64:65], ones_r[:].unsqueeze(2))

        # ---- per head-pair: Q.T projection, then attention ----
        for mt in range(OD // P):
            for n in range(2):
                csl = slice(n * 1024, (n + 1) * 1024)
                ps = psum_pool.tile([P, 1024], F32, tag="lg", name="psq")
                for i in range(NDC):
                    for j in range(2):
                        nc.tensor.matmul(
                            ps[:, j * 512:(j + 1) * 512],
                            wq_sb[i][:, mt * P:(mt + 1) * P],
                            x_sb[n][i][:, j * 512:(j + 1) * 512],
                            start=(i == 0), stop=(i == NDC - 1),
                        )
                nc.vector.tensor_scalar_add(
                    qT[mt][:, csl], ps[:], bq_sb[:, mt:mt + 1])

            for h in (2 * mt, 2 * mt + 1):
                ro = 64 * (h % 2)
                for qh in range(2):
                    q_sl = slice(qh * 1024, (qh + 1) * 1024)
                    av = psum_pool.tile([P, 1024], F32, tag="av", name="av")
                    for kc in range(NKC):
                        lg = psum_pool.tile([P, 1024], F32, tag="lg", name="lg")
                        for j in range(2):
                            nc.tensor.matmul(
                                lg[:, j * 512:(j + 1) * 512],
                                kT[h][:, kc * P:(kc + 1) * P],
                                qT[mt][:, qh * 1024 + j * 512:
                                        qh * 1024 + (j + 1) * 512],
                                start=True, stop=True,
                            )
                        es = e_pool.tile([P, 1024], F16, tag="es")
                        nc.scalar.activation(
                            es[:], lg[:], AF.Exp,
                            bias=mk_sb[:, kc:kc + 1], scale=0.125,
                        )
                        va_flat = v_aug[kc][:].rearrange("p h d -> p (h d)")
                        for j in range(2):
                            nc.tensor.matmul(
                                av[:, j * 512:(j + 1) * 512],
                                va_flat[:, 65 * h:65 * h + 128],
                                es[:, j * 512:(j + 1) * 512],
                                start=(kc == 0), stop=(kc == NKC - 1),
                            )
                    den = n_pool.tile([1, 1024], F32, tag="dn")
                    rec = n_pool.tile([1, 1024], F32, tag="r0")
                    bc = n_pool.tile([64, 1024], F32, tag="bc")
                    nc.vector.tensor_copy(den[:], av[64:65, :])
                    nc.vector.reciprocal_approx_fast(rec[:], den[:])
                    nc.gpsimd.partition_broadcast(bc[:], rec[:])
                    nc.vector.tensor_mul(
                        attn[mt][ro:ro + 64, q_sl], av[0:64, :], bc[:])

        # ---- out.T = wo_t.T @ attn.T (+bo_eff on core g=0) ----
        for m in range(D // P):
            for n in range(2):
                csl = slice(n * 1024, (n + 1) * 1024)
                ps = psum_pool.tile([P, 1024], F32, tag="av", name="pso")
                for i in range(OD // P):
                    for j in range(2):
                        nc.tensor.matmul(
                            ps[:, j * 512:(j + 1) * 512],
                            wo_sb[i][:, m * P:(m + 1) * P],
                            attn[i][:, n * 1024 + j * 512:
                                    n * 1024 + (j + 1) * 512],
                            start=(i == 0), stop=(i == OD // P - 1),
                        )
                ev = o_pool.tile([P, 1024], F16, tag="ev")
                if n == 0:
                    nc.vector.tensor_scalar_add(ev[:], ps[:], bo_sb[:, m:m + 1])
                else:
                    nc.scalar.activation(
                        ev[:], ps[:], AF.Identity, bias=bo_sb[:, m:m + 1])
                eng = nc.sync if n == 0 else nc.gpsimd
                eng.dma_start(out=out_t[m * P:(m + 1) * P, csl], in_=ev[:])

    nc.compile()
    return nc


def _prep_inputs(x, memory, mask, wq, bq, wk, bk, wv, bv, wo, bo):
    f = np.float32
    h = np.float16
    wqT = np.ascontiguousarray(wq.T, dtype=f)
    wkT = np.ascontiguousarray(wk.T, dtype=f)
    wvT = np.ascontiguousarray(wv.T, dtype=f)
    woT = np.ascontiguousarray(wo.T, dtype=f)
    bo_eff = (bo.astype(f) + wo.astype(f) @ bv.astype(f))
    zeros_bo = np.zeros_like(bo_eff)
    in_maps = []
    for c in range(NCORES):
        b, g = divmod(c, 2)
        sl = slice(g * OD, (g + 1) * OD)
        bo_c = bo_eff if g == 0 else zeros_bo
        in_maps.append({
            "x_t": np.ascontiguousarray(x[b].T, dtype=h),
            "mem_t": np.ascontiguousarray(memory[b].T, dtype=h),
            "wq_t": np.ascontiguousarray(wqT[:, sl]).astype(h),
            "wk_t": np.ascontiguousarray(wkT[:, sl]).astype(h),
            "wv_t": np.ascontiguousarray(wvT[:, sl]).astype(h),
            "wo_t": np.ascontiguousarray(woT[sl, :]).astype(h),
            "bq_s": np.ascontiguousarray(bq[sl].astype(f).reshape(OD // P, P).T),
            "bk_s": np.ascontiguousarray(bk[sl].astype(f).reshape(OD // P, P).T),
            "bo_s": np.ascontiguousarray(bo_c.reshape(D // P, P).T),
            "maskb": np.ascontiguousarray(
                np.where(mask[b], np.float32(NEG), np.float32(0.0))
                .astype(f).reshape(NKC, P).T),
        })
    return in_maps


def kernel(x, memory, mask, wq, bq, wk, bk, wv, bv, wo, bo, **run_kwargs):
    x = np.asarray(x, dtype=np.float32)
    memory = np.asarray(memory, dtype=np.float32)
    mask = np.asarray(mask)
    if "nc" not in _cache:
        _cache["nc"] = _build()
    nc = _cache["nc"]
    in_maps = _prep_inputs(x, memory, mask, wq, bq, wk, bk, wv, bv, wo, bo)
    res = run_bass_kernel_spmd(nc, in_maps, list(range(NCORES)), **run_kwargs)
    out = np.empty((B, S, D), dtype=np.float32)
    for b in range(B):
        part = (res.results[2 * b]["out_t"].astype(np.float32)
                + res.results[2 * b + 1]["out_t"].astype(np.float32))
        out[b] = part.T
    if run_kwargs:
        _cache["last_results"] = res
    return out


# revision 13
# speedup vs baseline: 1.0591x; 1.0085x over previous
"""Multi-head cross-attention (B=4, S=2048, D=1024, H=16) on 8 Trainium2 cores.

Sharding: hybrid data/tensor parallel. Core c handles batch b = c//2 and
head-group g = c%2 (8 of the 16 heads, i.e. 512 of the 1024 q/k/v dims).
Each core computes a partial out-projection over its 512 attention dims;
the host sums the two partials per batch.

v3: software-pipelined single-core schedule. The attention stream
(logits -> exp -> AV) is ACT-bound per chunk, so projection matmul
"filler" units are interleaved into the attention blocks to keep the
PE dense:
  - prologue: K-proj(m=0), Q-proj(mt=0), V(st=0)
  - block (mt0,h0,qh0): V(st=1..15) emitted just-in-time
  - later blocks: K(m=mt+1) / Q(mt+1) fillers; out-proj n=0 fillers in
    the last two blocks; rest of out-proj in the epilogue
PSUM: lg ring x2 (4 banks) + av x1 (2) + proj x1 (2). The av psum is
freed immediately by a copy to SBUF; normalization (reciprocal +
partition broadcast + multiply) runs off the critical path from SBUF.
Within a block, lg(kc) matmuls are emitted before av(kc-1) so the PE
never head-of-line blocks on the exp of the current chunk.
"""

import numpy as np

import concourse.bacc as bacc
import concourse.mybir as mybir
from concourse import tile
from concourse.bass_utils import run_bass_kernel_spmd

F32 = mybir.dt.float32
F16 = mybir.dt.float16
AF = mybir.ActivationFunctionType

B, S, D = 4, 2048, 1024
H, HD = 16, 64
NCORES = 8
NH = 8          # heads per core
OD = NH * HD    # 512 attention dims per core
P = 128
NDC = D // P    # 8 d-chunks
NKC = S // P    # 16 key chunks
NEG = -1.0e30

_cache = {}


def _build():
    from contextlib import ExitStack

    nc = bacc.Bacc(None, target_bir_lowering=False, debug=False)

    x_t = nc.dram_tensor("x_t", [D, S], F16, kind="ExternalInput").ap()
    mem_t = nc.dram_tensor("mem_t", [D, S], F16, kind="ExternalInput").ap()
    wq_t = nc.dram_tensor("wq_t", [D, OD], F16, kind="ExternalInput").ap()
    wk_t = nc.dram_tensor("wk_t", [D, OD], F16, kind="ExternalInput").ap()
    wv_t = nc.dram_tensor("wv_t", [D, OD], F16, kind="ExternalInput").ap()
    wo_t = nc.dram_tensor("wo_t", [OD, D], F16, kind="ExternalInput").ap()
    bq_s = nc.dram_tensor("bq_s", [P, OD // P], F32, kind="ExternalInput").ap()
    bk_s = nc.dram_tensor("bk_s", [P, OD // P], F32, kind="ExternalInput").ap()
    bo_s = nc.dram_tensor("bo_s", [P, D // P], F32, kind="ExternalInput").ap()
    maskb = nc.dram_tensor("maskb", [P, NKC], F32, kind="ExternalInput").ap()
    out_t = nc.dram_tensor("out_t", [D, S], F16, kind="ExternalOutput").ap()

    x_c = x_t.rearrange("(c p) s -> c p s", p=P)
    m_c = mem_t.rearrange("(c p) s -> c p s", p=P)
    wq_c = wq_t.rearrange("(c p) o -> c p o", p=P)
    wk_c = wk_t.rearrange("(c p) o -> c p o", p=P)
    wv_c = wv_t.rearrange("(c p) o -> c p o", p=P)
    wo_c = wo_t.rearrange("(c p) o -> c p o", p=P)

    with tile.TileContext(nc) as tc, ExitStack() as ctx:
        q_pool = ctx.enter_context(tc.tile_pool(name="qt", bufs=1))
        k_pool = ctx.enter_context(tc.tile_pool(name="kt", bufs=1))
        v_pool = ctx.enter_context(tc.tile_pool(name="va", bufs=1))
        a_pool = ctx.enter_context(tc.tile_pool(name="at", bufs=1))
        c_pool = ctx.enter_context(tc.tile_pool(name="cst", bufs=1))
        w_pool = ctx.enter_context(tc.tile_pool(name="wt", bufs=1))
        e_pool = ctx.enter_context(tc.tile_pool(name="es", bufs=4))
        n_pool = ctx.enter_context(tc.tile_pool(name="nrm", bufs=1))
        o_pool = ctx.enter_context(tc.tile_pool(name="ev", bufs=2))
        s_pool = ctx.enter_context(tc.tile_pool(name="avs", bufs=2))
        lg_pool = ctx.enter_context(tc.tile_pool(name="plg", bufs=2, space="PSUM"))
        av_pool = ctx.enter_context(tc.tile_pool(name="pav", bufs=1, space="PSUM"))
        pj_pool = ctx.enter_context(tc.tile_pool(name="ppj", bufs=1, space="PSUM"))
        xm_pool = ctx.enter_context(tc.tile_pool(name="xm", bufs=32))

        # ---- weight / input DMAs: spread across queues, first-needed first
        wk_sb = [w_pool.tile([P, OD], F16, tag="wk", name=f"wk{i}", bufs=NDC)
                 for i in range(NDC)]
        for i in range(NDC):
            nc.sync.dma_start(out=wk_sb[i][:], in_=wk_c[i])

        m_sb = [[xm_pool.tile([P, 1024], F16, tag="xm", name=f"m{n}_{i}")
                 for i in range(NDC)] for n in range(2)]
        for i in range(NDC):
            nc.gpsimd.dma_start(out=m_sb[0][i][:], in_=m_c[i, :, 0:1024])
        bq_sb = c_pool.tile([P, OD // P], F32, tag="bq")
        bk_sb = c_pool.tile([P, OD // P], F32, tag="bk")
        bo_sb = c_pool.tile([P, D // P], F32, tag="bo")
        mk_sb = c_pool.tile([P, NKC], F32, tag="mk")
        nc.scalar.dma_start(out=bk_sb[:], in_=bk_s[:])
        for i in range(NDC):
            nc.scalar.dma_start(out=m_sb[1][i][:], in_=m_c[i, :, 1024:2048])
        nc.scalar.dma_start(out=bq_sb[:], in_=bq_s[:])
        nc.scalar.dma_start(out=mk_sb[:], in_=maskb[:])
        nc.scalar.dma_start(out=bo_sb[:], in_=bo_s[:])

        wq_sb = [w_pool.tile([P, OD], F16, tag="wq", name=f"wq{i}", bufs=NDC)
                 for i in range(NDC)]
        for i in range(NDC):
            nc.sync.dma_start(out=wq_sb[i][:], in_=wq_c[i])
        wv_sb = [w_pool.tile([P, OD], F16, tag="wv", name=f"wv{i}", bufs=NDC)
                 for i in range(NDC)]
        for i in range(NDC):
            nc.sync.dma_start(out=wv_sb[i][:], in_=wv_c[i])
        wo_sb = [w_pool.tile([P, D], F16, tag="wo", name=f"wo{i}", bufs=OD // P)
                 for i in range(OD // P)]
        for i in range(OD // P):
            nc.sync.dma_start(out=wo_sb[i][:], in_=wo_c[i])

        x_sb = [[xm_pool.tile([P, 1024], F16, tag="xm", name=f"x{n}_{i}")
                 for i in range(NDC)] for n in range(2)]
        for n in range(2):
            for i in range(NDC):
                nc.gpsimd.dma_start(out=x_sb[n][i][:],
                                    in_=x_c[i, :, n * 1024:(n + 1) * 1024])

        # ---- persistent tiles ----
        qT = [q_pool.tile([P, S], F16, tag=f"q{m}", name=f"q{m}")
              for m in range(OD // P)]
        # packed K: rows 0:64 = head 2m, rows 64:128 = head 2m+1; the
        # logits matmul contracts only the 64 rows of its head
        kP = [k_pool.tile([P, S], F16, tag=f"k{m}", name=f"k{m}")
              for m in range(OD // P)]
        ones_f = c_pool.tile([P, NH], F32, tag="onef")
        nc.vector.memset(ones_f[:], 1.0)
        ones_r = c_pool.tile([P, NH], F16, tag="oner")
        nc.vector.tensor_copy(ones_r[:], ones_f[:])
        v_aug = [v_pool.tile([P, 9, 65], F16, tag=f"v{st}", name=f"v{st}")
                 for st in range(NKC)]
        for st in range(NKC):
            nc.vector.memset(v_aug[st][:, 8, :], 0.0)
        # attn tiles split by query half for precise out-proj deps
        attn = [[a_pool.tile([P, 1024], F16, tag=f"a{n}_{m}", name=f"a{n}_{m}")
                 for m in range(OD // P)] for n in range(2)]

        # ---------- filler unit machinery ----------
        def kproj_units(m):
            units = []
            for n in range(2):
                state = {}
                csl = slice(n * 1024, (n + 1) * 1024)
                for part in range(4):
                    def u(m=m, n=n, part=part, state=state, csl=csl):
                        if part == 0:
                            state["ps"] = pj_pool.tile(
                                [P, 1024], F32, tag="pj", name=f"pk{m}{n}")
                        ps = state["ps"]
                        for i in range(part * 2, part * 2 + 2):
                            for j in range(2):
                                nc.tensor.matmul(
                                    ps[:, j * 512:(j + 1) * 512],
                                    wk_sb[i][:, m * P:(m + 1) * P],
                                    m_sb[n][i][:, j * 512:(j + 1) * 512],
                                    start=(i == 0), stop=(i == NDC - 1),
                                )
                        if part == 3:
                            nc.vector.tensor_scalar_add(
                                kP[m][:, csl], ps[:], bk_sb[:, m:m + 1])
                    units.append(u)
            return units

        def qproj_units(mt):
            units = []
            for n in range(2):
                state = {}
                csl = slice(n * 1024, (n + 1) * 1024)
                for part in range(4):
                    def u(mt=mt, n=n, part=part, state=state, csl=csl):
                        if part == 0:
                            state["ps"] = pj_pool.tile(
                                [P, 1024], F32, tag="pj", name=f"pq{mt}{n}")
                        ps = state["ps"]
                        for i in range(part * 2, part * 2 + 2):
                            for j in range(2):
                                nc.tensor.matmul(
                                    ps[:, j * 512:(j + 1) * 512],
                                    wq_sb[i][:, mt * P:(mt + 1) * P],
                                    x_sb[n][i][:, j * 512:(j + 1) * 512],
                                    start=(i == 0), stop=(i == NDC - 1),
                                )
                        if part == 3:
                            nc.vector.tensor_scalar_add(
                                qT[mt][:, csl], ps[:], bq_sb[:, mt:mt + 1])
                    units.append(u)
            return units

        def vproj_units(st):
            units = []
            state = {}
            n, sc = divmod(st, 8)
            for part in range(2):
                def u(st=st, n=n, sc=sc, part=part, state=state):
                    if part == 0:
                        state["ps"] = pj_pool.tile(
                            [P, 1024], F32, tag="pj", name=f"pv{st}")
                    ps = state["ps"]
                    for i in range(part * 4, part * 4 + 4):
                        nc.tensor.matmul(
                            ps[:, 0:OD], m_sb[n][i][:, sc * P:(sc + 1) * P],
                            wv_sb[i][:],
                            start=(i == 0), stop=(i == NDC - 1),
                        )
                    if part == 1:
                        nc.vector.tensor_copy(
                            v_aug[st][:, 0:NH, 0:64],
                            ps[:, 0:OD].rearrange("p (h d) -> p h d", h=NH),
                        )
                        nc.vector.tensor_copy(
                            v_aug[st][:, 0:NH, 64:65], ones_r[:].unsqueeze(2))
                units.append(u)
            return units

        def oproj_units(m, n):
            units = []
            state = {}
            csl = slice(n * 1024, (n + 1) * 1024)
            for part in range(2):
                def u(m=m, n=n, part=part, state=state, csl=csl):
                    if part == 0:
                        state["ps"] = pj_pool.tile(
                            [P, 1024], F32, tag="pj", name=f"po{m}{n}")
                    ps = state["ps"]
                    for i in range(part * 2, part * 2 + 2):
                        for j in range(2):
                            nc.tensor.matmul(
                                ps[:, j * 512:(j + 1) * 512],
                                wo_sb[i][:, m * P:(m + 1) * P],
                                attn[n][i][:, j * 512:(j + 1) * 512],
                                start=(i == 0), stop=(i == OD // P - 1),
                            )
                    if part == 1:
                        ev = o_pool.tile([P, 1024], F16, tag="ev")
                        if m % 2 == 0:
                            nc.vector.tensor_scalar_add(
                                ev[:], ps[:], bo_sb[:, m:m + 1])
                        else:
                            nc.scalar.activation(
                                ev[:], ps[:], AF.Identity,
                                bias=bo_sb[:, m:m + 1])
                        nc.sync.dma_start(
                            out=out_t[m * P:(m + 1) * P, csl], in_=ev[:])
                units.append(u)
            return units

        fillers = []   # drained inside attention blocks

        def attention_block(mt, h, qh, jit_v=False, rate=4):
            """One (head, query-half) attention block, software pipelined.

            jit_v: emit V-projection units just-in-time (first block only).
            rate: drain one filler unit every `rate` chunks.
            """
            ro = 64 * (h % 2)
            av = av_pool.tile([P, 1024], F32, tag="av", name="av")
            es_tiles = {}
            for kc in range(NKC):
                if jit_v and kc + 1 < NKC:
                    for u in vproj_units(kc + 1):
                        u()
                lg = lg_pool.tile([P, 1024], F32, tag="lg", name="lg")
                for j in range(2):
                    nc.tensor.matmul(
                        lg[:, j * 512:(j + 1) * 512],
                        kP[mt][ro:ro + 64, kc * P:(kc + 1) * P],
                        qT[mt][ro:ro + 64, qh * 1024 + j * 512:
                               qh * 1024 + (j + 1) * 512],
                        start=True, stop=True,
                    )
                es = e_pool.tile([P, 1024], F16, tag="es")
                nc.scalar.activation(
                    es[:], lg[:], AF.Exp,
                    bias=mk_sb[:, kc:kc + 1], scale=0.125,
                )
                es_tiles[kc] = es
                # AV for the previous chunk (keeps PE ahead of ACT)
                if kc >= 1:
                    _av_mm(av, h, kc - 1, es_tiles.pop(kc - 1))
                if (not jit_v) and fillers and kc % rate == rate - 1:
                    fillers.pop(0)[1]()
            _av_mm(av, h, NKC - 1, es_tiles.pop(NKC - 1), last=True)
            # free the av psum quickly, normalize from SBUF.
            # NB: reciprocal_approx_fast (custom DVE op) only works on APs
            # based at partition 0 — stage the denominator row there first.
            avs = s_pool.tile([65, 1024], F32, tag="avs")
            nc.vector.tensor_copy(avs[:], av[0:65, :])
            den = n_pool.tile([1, 1024], F32, tag="dn")
            rec = n_pool.tile([1, 1024], F32, tag="r0")
            bc = n_pool.tile([64, 1024], F32, tag="bc")
            nc.vector.tensor_copy(den[:], avs[64:65, :])
            nc.vector.reciprocal_approx_fast(rec[:], den[:])
            nc.gpsimd.partition_broadcast(bc[:], rec[:])
            nc.vector.tensor_mul(
                attn[qh][mt][ro:ro + 64, :], avs[0:64, :], bc[:])

        def _av_mm(av, h, kc, es, last=False):
            va_flat = v_aug[kc][:].rearrange("p h d -> p (h d)")
            for j in range(2):
                nc.tensor.matmul(
                    av[:, j * 512:(j + 1) * 512],
                    va_flat[:, 65 * h:65 * h + 128],
                    es[:, j * 512:(j + 1) * 512],
                    start=(kc == 0), stop=last,
                )

        # ---------- prologue: K(m0), V(st0), Q(mt0) dense ----------
        for u in kproj_units(0):
            u()
        for u in vproj_units(0):
            u()
        for u in qproj_units(0):
            u()

        # ---------- attention blocks ----------
        # first block carries the V projection just-in-time
        attention_block(0, 0, 0, jit_v=True)
        # queue K/Q for later head-pairs as fillers; tag = the mt whose
        # blocks require the unit to have been emitted (barrier below)
        for m in range(1, OD // P):
            for u in kproj_units(m):
                fillers.append((m, u))
            for u in qproj_units(m):
                fillers.append((m, u))

        order = []
        for mt in range(OD // P):
            for qh in range(2):
                for h in (2 * mt, 2 * mt + 1):
                    if (mt, h, qh) != (0, 0, 0):
                        order.append((mt, h, qh))

        for mt, h, qh in order:
            # barrier: everything this mt needs must be emitted first
            while fillers and fillers[0][0] <= mt:
                fillers.pop(0)[1]()
            # out-proj n=0 becomes available once all qh=0 attn written
            if (mt, h, qh) == (3, 6, 1):
                for m in range(D // P):
                    for u in oproj_units(m, 0):
                        fillers.append((4, u))
            rate = 4 if fillers else NKC + 1
            attention_block(mt, h, qh, rate=rate)

        # ---------- epilogue: drain remaining fillers, then out n=1 ----
        while fillers:
            fillers.pop(0)[1]()
        for m in range(D // P):
            for u in oproj_units(m, 1):
                u()

    nc.compile()
    return nc


def _prep_inputs(x, memory, mask, wq, bq, wk, bk, wv, bv, wo, bo):
    f = np.float32
    h = np.float16
    wqT = np.ascontiguousarray(wq.T, dtype=f)
    wkT = np.ascontiguousarray(wk.T, dtype=f)
    wvT = np.ascontiguousarray(wv.T, dtype=f)
    woT = np.ascontiguousarray(wo.T, dtype=f)
    bo_eff = (bo.astype(f) + wo.astype(f) @ bv.astype(f))
    zeros_bo = np.zeros_like(bo_eff)
    in_maps = []
    for c in range(NCORES):
        b, g = divmod(c, 2)
        sl = slice(g * OD, (g + 1) * OD)
        bo_c = bo_eff if g == 0 else zeros_bo
        in_maps.append({
            "x_t": np.ascontiguousarray(x[b].T, dtype=h),
            "mem_t": np.ascontiguousarray(memory[b].T, dtype=h),
            "wq_t": np.ascontiguousarray(wqT[:, sl]).astype(h),
            "wk_t": np.ascontiguousarray(wkT[:, sl]).astype(h),
            "wv_t": np.ascontiguousarray(wvT[:, sl]).astype(h),
            "wo_t": np.ascontiguousarray(woT[sl, :]).astype(h),
            "bq_s": np.ascontiguousarray(bq[sl].astype(f).reshape(OD // P, P).T),
            "bk_s": np.ascontiguousarray(bk[sl].astype(f).reshape(OD // P, P).T),
            "bo_s": np.ascontiguousarray(bo_c.reshape(D // P, P).T),
            "maskb": np.ascontiguousarray(
                np.where(mask[b], np.float32(NEG), np.float32(0.0))
                .astype(f).reshape(NKC, P).T),
        })
    return in_maps


def kernel(x, memory, mask, wq, bq, wk, bk, wv, bv, wo, bo, **run_kwargs):
    x = np.asarray(x, dtype=np.float32)
    memory = np.asarray(memory, dtype=np.float32)
    mask = np.asarray(mask)
    if "nc" not in _cache:
        _cache["nc"] = _build()
    nc = _cache["nc"]
    in_maps = _prep_inputs(x, memory, mask, wq, bq, wk, bk, wv, bv, wo, bo)
    res = run_bass_kernel_spmd(nc, in_maps, list(range(NCORES)), **run_kwargs)
    out = np.empty((B, S, D), dtype=np.float32)
    for b in range(B):
        part = (res.results[2 * b]["out_t"].astype(np.float32)
                + res.results[2 * b + 1]["out_t"].astype(np.float32))
        out[b] = part.T
    if run_kwargs:
        _cache["last_results"] = res
    return out


# revision 19
# speedup vs baseline: 1.0696x; 1.0098x over previous
"""Multi-head cross-attention (B=4, S=2048, D=1024, H=16) on 8 Trainium2 cores.

Sharding: hybrid data/tensor parallel. Core c handles batch b = c//2 and
head-group g = c%2 (8 of the 16 heads, i.e. 512 of the 1024 q/k/v dims).
Each core computes a partial out-projection over its 512 attention dims;
the host sums the two partials per batch.

v3: software-pipelined single-core schedule. The attention stream
(logits -> exp -> AV) is ACT-bound per chunk, so projection matmul
"filler" units are interleaved into the attention blocks to keep the
PE dense:
  - prologue: K-proj(m=0), Q-proj(mt=0), V(st=0)
  - block (mt0,h0,qh0): V(st=1..15) emitted just-in-time
  - later blocks: K(m=mt+1) / Q(mt+1) fillers; out-proj n=0 fillers in
    the last two blocks; rest of out-proj in the epilogue
PSUM: lg ring x2 (4 banks) + av x1 (2) + proj x1 (2). The av psum is
freed immediately by a copy to SBUF; normalization (reciprocal +
partition broadcast + multiply) runs off the critical path from SBUF.
Within a block, lg(kc) matmuls are emitted before av(kc-1) so the PE
never head-of-line blocks on the exp of the current chunk.
"""

import numpy as np

import concourse.bacc as bacc
import concourse.mybir as mybir
from concourse import tile
from concourse.bass_utils import run_bass_kernel_spmd

F32 = mybir.dt.float32
F16 = mybir.dt.float16
AF = mybir.ActivationFunctionType

B, S, D = 4, 2048, 1024
H, HD = 16, 64
NCORES = 8
NH = 8          # heads per core
OD = NH * HD    # 512 attention dims per core
P = 128
NDC = D // P    # 8 d-chunks
NKC = S // P    # 16 key chunks
NEG = -1.0e30

_cache = {}


def _build():
    from contextlib import ExitStack

    nc = bacc.Bacc(None, target_bir_lowering=False, debug=False)

    x_t = nc.dram_tensor("x_t", [D, S], F16, kind="ExternalInput").ap()
    mem_t = nc.dram_tensor("mem_t", [D, S], F16, kind="ExternalInput").ap()
    wq_t = nc.dram_tensor("wq_t", [D, OD], F16, kind="ExternalInput").ap()
    wk_t = nc.dram_tensor("wk_t", [D, OD], F16, kind="ExternalInput").ap()
    wv_t = nc.dram_tensor("wv_t", [D, OD], F16, kind="ExternalInput").ap()
    wo_t = nc.dram_tensor("wo_t", [OD, D], F16, kind="ExternalInput").ap()
    bq_s = nc.dram_tensor("bq_s", [P, OD // P], F32, kind="ExternalInput").ap()
    bk_s = nc.dram_tensor("bk_s", [P, OD // P], F32, kind="ExternalInput").ap()
    bo_s = nc.dram_tensor("bo_s", [P, D // P], F32, kind="ExternalInput").ap()
    maskb = nc.dram_tensor("maskb", [P, NKC], F32, kind="ExternalInput").ap()
    out_t = nc.dram_tensor("out_t", [D, S], F16, kind="ExternalOutput").ap()

    x_c = x_t.rearrange("(c p) s -> c p s", p=P)
    m_c = mem_t.rearrange("(c p) s -> c p s", p=P)
    wq_c = wq_t.rearrange("(c p) o -> c p o", p=P)
    wk_c = wk_t.rearrange("(c p) o -> c p o", p=P)
    wv_c = wv_t.rearrange("(c p) o -> c p o", p=P)
    wo_c = wo_t.rearrange("(c p) o -> c p o", p=P)

    with tile.TileContext(nc) as tc, ExitStack() as ctx:
        q_pool = ctx.enter_context(tc.tile_pool(name="qt", bufs=1))
        k_pool = ctx.enter_context(tc.tile_pool(name="kt", bufs=1))
        v_pool = ctx.enter_context(tc.tile_pool(name="va", bufs=1))
        a_pool = ctx.enter_context(tc.tile_pool(name="at", bufs=1))
        c_pool = ctx.enter_context(tc.tile_pool(name="cst", bufs=1))
        w_pool = ctx.enter_context(tc.tile_pool(name="wt", bufs=1))
        e_pool = ctx.enter_context(tc.tile_pool(name="es", bufs=6))
        n_pool = ctx.enter_context(tc.tile_pool(name="nrm", bufs=1))
        o_pool = ctx.enter_context(tc.tile_pool(name="ev", bufs=2))
        s_pool = ctx.enter_context(tc.tile_pool(name="avs", bufs=2))
        lg_pool = ctx.enter_context(tc.tile_pool(name="plg", bufs=2, space="PSUM"))
        av_pool = ctx.enter_context(tc.tile_pool(name="pav", bufs=1, space="PSUM"))
        pj_pool = ctx.enter_context(tc.tile_pool(name="ppj", bufs=1, space="PSUM"))
        xm_pool = ctx.enter_context(tc.tile_pool(name="xm", bufs=32))

        # ---- weight / input DMAs: spread across queues, first-needed first
        wk_sb = [w_pool.tile([P, OD], F16, tag="wk", name=f"wk{i}", bufs=NDC)
                 for i in range(NDC)]
        for i in range(NDC):
            nc.sync.dma_start(out=wk_sb[i][:], in_=wk_c[i])

        m_sb = [[xm_pool.tile([P, 1024], F16, tag="xm", name=f"m{n}_{i}")
                 for i in range(NDC)] for n in range(2)]
        for i in range(NDC):
            nc.gpsimd.dma_start(out=m_sb[0][i][:], in_=m_c[i, :, 0:1024])
        bq_sb = c_pool.tile([P, OD // P], F32, tag="bq")
        bk_sb = c_pool.tile([P, OD // P], F32, tag="bk")
        bo_sb = c_pool.tile([P, D // P], F32, tag="bo")
        mk_sb = c_pool.tile([P, NKC], F32, tag="mk")
        nc.scalar.dma_start(out=bk_sb[:], in_=bk_s[:])
        for i in range(NDC):
            nc.scalar.dma_start(out=m_sb[1][i][:], in_=m_c[i, :, 1024:2048])
        nc.scalar.dma_start(out=bq_sb[:], in_=bq_s[:])
        nc.scalar.dma_start(out=mk_sb[:], in_=maskb[:])
        nc.scalar.dma_start(out=bo_sb[:], in_=bo_s[:])

        wv_sb = [w_pool.tile([P, OD], F16, tag="wv", name=f"wv{i}", bufs=NDC)
                 for i in range(NDC)]
        for i in range(NDC):
            nc.sync.dma_start(out=wv_sb[i][:], in_=wv_c[i])
        wq_sb = [w_pool.tile([P, OD], F16, tag="wq", name=f"wq{i}", bufs=NDC)
                 for i in range(NDC)]
        for i in range(NDC):
            nc.sync.dma_start(out=wq_sb[i][:], in_=wq_c[i])
        wo_sb = [w_pool.tile([P, D], F16, tag="wo", name=f"wo{i}", bufs=OD // P)
                 for i in range(OD // P)]
        for i in range(OD // P):
            nc.sync.dma_start(out=wo_sb[i][:], in_=wo_c[i])

        x_sb = [[xm_pool.tile([P, 1024], F16, tag="xm", name=f"x{n}_{i}")
                 for i in range(NDC)] for n in range(2)]
        for n in range(2):
            for i in range(NDC):
                nc.gpsimd.dma_start(out=x_sb[n][i][:],
                                    in_=x_c[i, :, n * 1024:(n + 1) * 1024])

        # ---- persistent tiles ----
        qT = [q_pool.tile([P, S], F16, tag=f"q{m}", name=f"q{m}")
              for m in range(OD // P)]
        # packed K: rows 0:64 = head 2m, rows 64:128 = head 2m+1; the
        # logits matmul contracts only the 64 rows of its head
        kP = [k_pool.tile([P, S], F16, tag=f"k{m}", name=f"k{m}")
              for m in range(OD // P)]
        ones_f = c_pool.tile([P, NH], F32, tag="onef")
        nc.vector.memset(ones_f[:], 1.0)
        ones_r = c_pool.tile([P, NH], F16, tag="oner")
        nc.vector.tensor_copy(ones_r[:], ones_f[:])
        v_aug = [v_pool.tile([P, 9, 65], F16, tag=f"v{st}", name=f"v{st}")
                 for st in range(NKC)]
        for st in range(NKC):
            nc.vector.memset(v_aug[st][:, 8, :], 0.0)
        # attn tiles split by query half for precise out-proj deps
        attn = [[a_pool.tile([P, 1024], F16, tag=f"a{n}_{m}", name=f"a{n}_{m}")
                 for m in range(OD // P)] for n in range(2)]

        # ---------- filler unit machinery ----------
        def kproj_units(m):
            units = []
            for n in range(2):
                state = {}
                csl = slice(n * 1024, (n + 1) * 1024)
                for part in range(4):
                    def u(m=m, n=n, part=part, state=state, csl=csl):
                        if part == 0:
                            state["ps"] = pj_pool.tile(
                                [P, 1024], F32, tag="pj", name=f"pk{m}{n}")
                        ps = state["ps"]
                        for i in range(part * 2, part * 2 + 2):
                            for j in range(2):
                                nc.tensor.matmul(
                                    ps[:, j * 512:(j + 1) * 512],
                                    wk_sb[i][:, m * P:(m + 1) * P],
                                    m_sb[n][i][:, j * 512:(j + 1) * 512],
                                    start=(i == 0), stop=(i == NDC - 1),
                                )
                        if part == 3:
                            nc.vector.tensor_scalar_add(
                                kP[m][:, csl], ps[:], bk_sb[:, m:m + 1])
                    units.append(u)
            return units

        def qproj_units(mt, ns=(0, 1)):
            units = []
            for n in ns:
                state = {}
                csl = slice(n * 1024, (n + 1) * 1024)
                for part in range(4):
                    def u(mt=mt, n=n, part=part, state=state, csl=csl):
                        if part == 0:
                            state["ps"] = pj_pool.tile(
                                [P, 1024], F32, tag="pj", name=f"pq{mt}{n}")
                        ps = state["ps"]
                        for i in range(part * 2, part * 2 + 2):
                            for j in range(2):
                                nc.tensor.matmul(
                                    ps[:, j * 512:(j + 1) * 512],
                                    wq_sb[i][:, mt * P:(mt + 1) * P],
                                    x_sb[n][i][:, j * 512:(j + 1) * 512],
                                    start=(i == 0), stop=(i == NDC - 1),
                                )
                        if part == 3:
                            nc.vector.tensor_scalar_add(
                                qT[mt][:, csl], ps[:], bq_sb[:, mt:mt + 1])
                    units.append(u)
            return units

        def vproj_units(st):
            units = []
            state = {}
            n, sc = divmod(st, 8)
            for part in range(2):
                def u(st=st, n=n, sc=sc, part=part, state=state):
                    if part == 0:
                        state["ps"] = pj_pool.tile(
                            [P, 1024], F32, tag="pj", name=f"pv{st}")
                    ps = state["ps"]
                    for i in range(part * 4, part * 4 + 4):
                        nc.tensor.matmul(
                            ps[:, 0:OD], m_sb[n][i][:, sc * P:(sc + 1) * P],
                            wv_sb[i][:],
                            start=(i == 0), stop=(i == NDC - 1),
                        )
                    if part == 1:
                        nc.vector.tensor_copy(
                            v_aug[st][:, 0:NH, 0:64],
                            ps[:, 0:OD].rearrange("p (h d) -> p h d", h=NH),
                        )
                        nc.vector.tensor_copy(
                            v_aug[st][:, 0:NH, 64:65], ones_r[:].unsqueeze(2))
                units.append(u)
            return units

        def oproj_units(m, n, pool=None):
            units = []
            state = {}
            csl = slice(n * 1024, (n + 1) * 1024)
            psum = pool if pool is not None else pj_pool
            tag = "pj" if psum is pj_pool else "lg"
            for part in range(2):
                def u(m=m, n=n, part=part, state=state, csl=csl,
                      psum=psum, tag=tag):
                    if part == 0:
                        state["ps"] = psum.tile(
                            [P, 1024], F32, tag=tag, name=f"po{m}{n}")
                    ps = state["ps"]
                    for i in range(part * 2, part * 2 + 2):
                        for j in range(2):
                            nc.tensor.matmul(
                                ps[:, j * 512:(j + 1) * 512],
                                wo_sb[i][:, m * P:(m + 1) * P],
                                attn[n][i][:, j * 512:(j + 1) * 512],
                                start=(i == 0), stop=(i == OD // P - 1),
                            )
                    if part == 1:
                        ev = o_pool.tile([P, 1024], F16, tag="ev")
                        nc.vector.tensor_scalar_add(
                            ev[:], ps[:], bo_sb[:, m:m + 1])
                        nc.sync.dma_start(
                            out=out_t[m * P:(m + 1) * P, csl], in_=ev[:])
                units.append(u)
            return units

        fillers = []   # drained inside attention blocks

        def attention_block(mt, h, qh, jit_v=False, rate=4):
            """One (head, query-half) attention block, software pipelined.

            jit_v: emit V-projection units just-in-time (first block only).
            rate: drain one filler unit every `rate` chunks.
            """
            ro = 64 * (h % 2)
            av = av_pool.tile([P, 1024], F32, tag="av", name="av")
            es_tiles = {}
            for kc in range(NKC):
                if jit_v and kc + 1 < NKC:
                    for u in vproj_units(kc + 1):
                        u()
                lg = lg_pool.tile([P, 1024], F32, tag="lg", name="lg")
                for j in range(2):
                    nc.tensor.matmul(
                        lg[:, j * 512:(j + 1) * 512],
                        kP[mt][ro:ro + 64, kc * P:(kc + 1) * P],
                        qT[mt][ro:ro + 64, qh * 1024 + j * 512:
                               qh * 1024 + (j + 1) * 512],
                        start=True, stop=True,
                    )
                es = e_pool.tile([P, 1024], F16, tag="es")
                nc.scalar.activation(
                    es[:], lg[:], AF.Exp,
                    bias=mk_sb[:, kc:kc + 1], scale=0.125,
                )
                es_tiles[kc] = es
                # AV for the previous chunk (keeps PE ahead of ACT)
                if kc >= 1:
                    _av_mm(av, h, kc - 1, es_tiles.pop(kc - 1))
                if (not jit_v) and fillers and kc % rate == rate - 1:
                    fillers.pop(0)[1]()
            _av_mm(av, h, NKC - 1, es_tiles.pop(NKC - 1), last=True)
            # free the av psum quickly, normalize from SBUF.
            # NB: reciprocal_approx_fast (custom DVE op) only works on APs
            # based at partition 0 — stage the denominator row there first.
            avs = s_pool.tile([65, 1024], F32, tag="avs")
            nc.vector.tensor_copy(avs[:], av[0:65, :])
            den = n_pool.tile([1, 1024], F32, tag="dn")
            rec = n_pool.tile([1, 1024], F32, tag="r0")
            bc = n_pool.tile([64, 1024], F32, tag="bc")
            nc.vector.tensor_copy(den[:], avs[64:65, :])
            nc.vector.reciprocal_approx_fast(rec[:], den[:])
            nc.gpsimd.partition_broadcast(bc[:], rec[:])
            nc.vector.tensor_mul(
                attn[qh][mt][ro:ro + 64, :], avs[0:64, :], bc[:])

        def _av_mm(av, h, kc, es, last=False):
            va_flat = v_aug[kc][:].rearrange("p h d -> p (h d)")
            for j in range(2):
                nc.tensor.matmul(
                    av[:, j * 512:(j + 1) * 512],
                    va_flat[:, 65 * h:65 * h + 128],
                    es[:, j * 512:(j + 1) * 512],
                    start=(kc == 0), stop=last,
                )

        # ---------- prologue: K(m0), V(st0), Q(mt0 n=0) dense ----------
        for u in kproj_units(0):
            u()
        for u in vproj_units(0):
            u()
        for u in qproj_units(0, ns=(0,)):
            u()

        # ---------- attention blocks ----------
        # first block carries the V projection just-in-time
        attention_block(0, 0, 0, jit_v=True)
        # queue K/Q for later head-pairs as fillers; tag = the (mt, qh)
        # key before which the unit must have been emitted (barrier below)
        for u in qproj_units(0, ns=(1,)):
            fillers.append((1, u))
        for m in range(1, OD // P):
            for u in kproj_units(m):
                fillers.append((2 * m, u))
            for u in qproj_units(m, ns=(0,)):
                fillers.append((2 * m, u))
            for u in qproj_units(m, ns=(1,)):
                fillers.append((2 * m + 1, u))

        order = []
        for mt in range(OD // P):
            for qh in range(2):
                for h in (2 * mt, 2 * mt + 1):
                    if (mt, h, qh) != (0, 0, 0):
                        order.append((mt, h, qh))

        for mt, h, qh in order:
            # barrier: everything this block needs must be emitted first
            while fillers and fillers[0][0] <= 2 * mt + qh:
                fillers.pop(0)[1]()
            # out-proj n=0 becomes available once all qh=0 attn written
            if (mt, h, qh) == (3, 6, 1):
                for m in range(D // P):
                    for u in oproj_units(m, 0):
                        fillers.append((4, u))
            rate = 2 if (mt, qh) == (3, 1) else (4 if fillers else NKC + 1)
            attention_block(mt, h, qh, rate=rate)

        # ---------- epilogue ----------
        # drain leftovers, then out-proj n=1; pair groups across the two
        # free psum pools, part0s first, so one group's matmuls cover the
        # other's psum->ev->store close
        while fillers:
            fillers.pop(0)[1]()
        groups = [oproj_units(m, 1, pool=(pj_pool if m % 2 == 0 else lg_pool))
                  for m in range(D // P)]
        for g in range(0, D // P, 2):
            ga, gb = groups[g], groups[g + 1]
            ga[0]()
            gb[0]()
            ga[1]()
            gb[1]()

    nc.compile()
    return nc


def _prep_inputs(x, memory, mask, wq, bq, wk, bk, wv, bv, wo, bo):
    f = np.float32
    h = np.float16
    wqT = np.ascontiguousarray(wq.T, dtype=f)
    wkT = np.ascontiguousarray(wk.T, dtype=f)
    wvT = np.ascontiguousarray(wv.T, dtype=f)
    woT = np.ascontiguousarray(wo.T, dtype=f)
    bo_eff = (bo.astype(f) + wo.astype(f) @ bv.astype(f))
    zeros_bo = np.zeros_like(bo_eff)
    in_maps = []
    for c in range(NCORES):
        b, g = divmod(c, 2)
        sl = slice(g * OD, (g + 1) * OD)
        bo_c = bo_eff if g == 0 else zeros_bo
        in_maps.append({
            "x_t": np.ascontiguousarray(x[b].T, dtype=h),
            "mem_t": np.ascontiguousarray(memory[b].T, dtype=h),
            "wq_t": np.ascontiguousarray(wqT[:, sl]).astype(h),
            "wk_t": np.ascontiguousarray(wkT[:, sl]).astype(h),
            "wv_t": np.ascontiguousarray(wvT[:, sl]).astype(h),
            "wo_t": np.ascontiguousarray(woT[sl, :]).astype(h),
            "bq_s": np.ascontiguousarray(bq[sl].astype(f).reshape(OD // P, P).T),
            "bk_s": np.ascontiguousarray(bk[sl].astype(f).reshape(OD // P, P).T),
            "bo_s": np.ascontiguousarray(bo_c.reshape(D // P, P).T),
            "maskb": np.ascontiguousarray(
                np.where(mask[b], np.float32(NEG), np.float32(0.0))
                .astype(f).reshape(NKC, P).T),
        })
    return in_maps


def kernel(x, memory, mask, wq, bq, wk, bk, wv, bv, wo, bo, **run_kwargs):
    x = np.asarray(x, dtype=np.float32)
    memory = np.asarray(memory, dtype=np.float32)
    mask = np.asarray(mask)
    if "nc" not in _cache:
        _cache["nc"] = _build()
    nc = _cache["nc"]
    in_maps = _prep_inputs(x, memory, mask, wq, bq, wk, bk, wv, bv, wo, bo)
    res = run_bass_kernel_spmd(nc, in_maps, list(range(NCORES)), **run_kwargs)
    out = np.empty((B, S, D), dtype=np.float32)
    for b in range(B):
        part = (res.results[2 * b]["out_t"].astype(np.float32)
                + res.results[2 * b + 1]["out_t"].astype(np.float32))
        out[b] = part.T
    if run_kwargs:
        _cache["last_results"] = res
    return out
